# revision 11
# baseline (speedup 1.0000x reference)
"""Continuous Game-of-Life Trainium2 kernel (product-form, 2-sigmoid).

Reference computation (per batch image, cyclic 3x3 stencil):
    around = 8-neighbor sum of x (torus wrap), u = 10*around
    survive = sigmoid(u-15) * sigmoid(35-u)
    birth   = sigmoid(u-25) * sigmoid(35-u)
    out     = x*survive + (1-x)*birth

Math used here (max abs err ~5e-5 vs reference, fp64):
    E1 := sigmoid(10 - |u-25|)        # == survive (err <= sigmoid(-10))
    E2 := sigmoid(u-25)
    birth == E1*E2 (err ~5e-5), so
    out = E1 * (x + E2 - x*E2)

This needs only TWO sigmoid passes on the Scalar engine (the baseline
three-sigmoid form is ScalarE-bound at ~196us busy).  The remaining
work is spread to keep every engine under the ~4.4us/strip DMA floor:
  - TensorE: 8-neighbor sum via banded matmuls (as before).
  - abs pass w = |around-2.5|: split ScalarE (Abs activation, ~30%) /
    VectorE (tensor_scalar add+abs_max, ~70%; PSUM source runs 1x).
  - blend t = x + E2 - x*E2: one fused custom-DVE op (BLEND1_ANT).
  - out = E1*t: split VectorE (2x fp16) / GpSimd.
  - DMA in fp32->fp16 (SWDGE cast), out fp16.

Sharding: data-parallel over batch: 16 images -> 8 cores x 2 images.
Torus wrap is per-image so there is no cross-core halo.
"""

import numpy as np

B, H, W = 16, 2048, 2048
N_CORES = 8
B_PER = B // N_CORES  # 2 images per core
STRIDE = 126  # output rows per strip (128 input rows incl. halos)
N_STRIPS = (H + STRIDE - 1) // STRIDE  # 17
NBANKS = W // 512  # PSUM banks per strip

# work-split knobs (elements of the 2048-wide free dim)
ABS_ACT_W = 640  # abs columns done on ScalarE (rest on VectorE)
MUL_DVE_W = 1280  # final-mul columns done on VectorE (rest on GpSimd)
USE_CUSTOM_BLEND = True

_cached_nc = None
_custom_ops = None


def _register_custom_ops():
    """Register fused custom DVE ops at runtime.

    Same mechanism as editing dve_ops.py (the per-NEFF uop table is
    generated at compile time from OPS); the sha is computed here so the
    pin always matches this interpreter's lowering.

      BLEND1_ANT:    out = in0 + in1 - in0*in1
      ABS_SHIFT_ANT: out = |in0 + s0|   (walrus rejects abs_max on
                     TensorScalar, so plain TS cannot do an abs)
    """
    global _custom_ops
    if _custom_ops is not None:
        return _custom_ops
    import numpy as np

    from concourse import dve_ops
    from concourse.dve_spec import C0, Spec, Src0, Src1, Zero, lower, maxx
    from concourse.dve_uop import DveOpSpec

    def _mk(name, spec):
        if name in dve_ops._SUB_OPCODE_FOR_NAME:
            return next(op for op in dve_ops.OPS if op.name == name)
        shas = {
            ver: DveOpSpec(
                name=name, opcode=0, uops=lower(spec, ver=ver), rd1_en=True
            ).sha(ver)
            for ver in ("v3", "v4")
        }
        op = dve_ops.DveOp(name, spec, subdim=False, uops_sha=shas)
        row = dve_ops._CUSTOM_DVE_ROW_BASE + len(dve_ops.OPS)
        assert row < 0x20
        dve_ops.OPS.append(op)
        dve_ops._SUB_OPCODE_FOR_NAME[name] = row
        dve_ops.CUSTOM_DVE_SPECS[name] = spec
        return op

    blend = _mk(
        "BLEND1_ANT",
        Spec(
            body=Src0 + Src1 - Src0 * Src1,
            reference=lambda in0, in1, s0, s1, imm2: in0 + in1 - in0 * in1,
        ),
    )
    _y = Src0 + C0
    absshift = _mk(
        "ABS_SHIFT_ANT",
        Spec(
            body=maxx(_y, Zero - _y),
            reference=lambda in0, in1, s0, s1, imm2: np.abs(in0 + s0),
        ),
    )
    _attach_blend_2x(blend)
    _custom_ops = (blend, absshift)
    return _custom_ops


def _attach_blend_2x(op):
    """Hand-authored 2x_1p uop program for BLEND1_ANT.

    lower() only emits the 1x program; the engine's perf-mode slots are
    real hardware (control_table[table_ptr+mode]) and dve_table_gen writes
    them when DveOpSpec.uops_2x is set.  Program (stock TENSOR_TENSOR
    2x_1p pattern, hi result carried on delay lane 1):
      entry: alu-slot=a_lo, d0=b_lo, d1=a_hi, d2=b_hi
      S0 s_hi=a_hi+b_hi (a_lo->d3)   S3 s_lo=a_lo+b_lo (t_hi->d1)
      S1 m_hi=a_hi*b_hi (s_hi->d4)   S4 m_lo=a_lo*b_lo (s_lo->d2)
      S2 t_hi=s_hi-m_hi              S5 t_lo=s_lo-m_lo
      write: WR0_LO<-ALU_OUT (t_lo), WR0_HI<-DELAY_1 (t_hi)
    Verified bit-exact vs the 1x program on hardware (test_blend2x.py).
    """
    from concourse import dve_ops
    from concourse.dve_spec import lower
    from concourse.dve_uop import (
        AluInp as A,
        AluOp as U,
        DelayInp,
        DveOpSpec,
        InpSel,
        OutPath,
        OutSel,
        Trigger,
        UopConfig,
        UopDpConfig,
    )

    PD = DelayInp.PREV_DELAY
    PAO = DelayInp.PREV_ALU_OUT

    def dp(opc, s0, s1, delays):
        d = [PD] * 7
        de = [0] * 7
        for lane, src in delays.items():
            d[lane] = src
            de[lane] = 1
        return UopDpConfig(
            op=opc, alu_src0=s0, alu_src1=s1, delay=d, delay_enable=de,
            alu_out_enable=1,
        )

    stages = [
        dp(U.ADD, A.PREV_DELAY_1, A.PREV_DELAY_2, {0: PD, 1: PD, 2: PD, 3: PAO}),
        dp(U.MULTIPLY, A.PREV_DELAY_1, A.PREV_DELAY_2, {0: PD, 3: PD, 4: PAO}),
        dp(U.SUBTRACT, A.PREV_DELAY_4, A.PREV_ALU_OUT, {0: PD, 3: PD}),
        dp(U.ADD, A.PREV_DELAY_3, A.PREV_DELAY_0, {0: PD, 1: PAO, 3: PD}),
        dp(U.MULTIPLY, A.PREV_DELAY_3, A.PREV_DELAY_0, {1: PD, 2: PAO}),
        dp(U.SUBTRACT, A.PREV_DELAY_2, A.PREV_ALU_OUT, {1: PD}),
        dp(U.BYPASS, A.PREV_ALU_OUT, A.PREV_ALU_OUT, {1: PD}),
        dp(U.BYPASS, A.PREV_ALU_OUT, A.PREV_ALU_OUT, {1: PD}),
    ]
    u2 = UopConfig(
        inp=[InpSel.SRC_0, InpSel.SRC_1, InpSel.SRC_0_HI, InpSel.SRC_1_HI]
        + [InpSel.ZERO] * 4,
        inp_enable=[1, 1, 1, 1, 0, 0, 0, 0],
        out={
            OutPath.WR0_LO: OutSel.ALU_OUT,
            OutPath.WR0_HI: OutSel.DELAY_1,
            OutPath.WR1_LO: OutSel.ALU_OUT,
            OutPath.WR1_HI: OutSel.ALU_OUT,
        },
        out_enable={
            OutPath.WR0_LO: 1, OutPath.WR0_HI: 1, OutPath.WR1_LO: 0, OutPath.WR1_HI: 0,
        },
        require_inp0=1,
        require_inp1=1,
        trigger=(Trigger.SRC_TENSOR_DONE, Trigger.NONE, Trigger.NONE),
        next_uop=(0, 0, 0),
        datapath_config=stages,
    )
    ver = "v3"
    u2.validate(ver)
    spec = DveOpSpec(
        name=op.name,
        opcode=dve_ops.get_dve_sub_opcode(op.name),
        uops=lower(op.spec, ver=ver),
        uops_2x=[u2],
        rd1_en=True,
        perf_max=1,
    )
    spec.validate(ver)
    dve_ops._COMPILE_CACHE[(op.name, ver)] = spec


def _band_matrices(m, dtype=np.float16):
    """[m+2, m] stationary operands for the vertical taps.

    Tile layout: partitions 0..m-1 hold image rows r0..r0+m-1 (the cells),
    partition m holds the bottom halo row r0+m, partition m+1 holds the top
    halo row r0-1.  For output row p the vertical neighbors are partitions
    p-1 (or m+1 when p==0) and p+1.

    m0[k, p] = 1 for the two vertical neighbors (no center),
    m1[k, p] = 1 for the full 3-tap (used on the column-shifted views).
    """
    m0 = np.zeros((m + 2, m), dtype)
    m1 = np.zeros((m + 2, m), dtype)
    for p in range(m):
        up = m + 1 if p == 0 else p - 1
        m0[up, p] = 1.0
        m0[p + 1, p] = 1.0
        m1[up, p] = 1.0
        m1[p, p] = 1.0
        m1[p + 1, p] = 1.0
    return m0, m1


def _build(b_per=B_PER, h=H, w=W, stride=STRIDE):
    global _cached_nc
    if _cached_nc is not None and (b_per, h, w, stride) == (B_PER, H, W, STRIDE):
        return _cached_nc

    import concourse.mybir as mybir
    from concourse.bacc import Bacc
    from concourse.tile import TileContext

    blend1, absshift = _register_custom_ops()

    B_PER_, H_, W_, STRIDE_ = b_per, h, w, stride
    N_STRIPS_ = (H_ + STRIDE_ - 1) // STRIDE_
    NBANKS_ = W_ // 512
    KROWS = STRIDE_ + 2

    f32 = mybir.dt.float32
    f16 = mybir.dt.float16
    Sig = mybir.ActivationFunctionType.Sigmoid
    AbsF = mybir.ActivationFunctionType.Abs
    Add = mybir.AluOpType.add
    AbsMax = mybir.AluOpType.abs_max

    nc = Bacc(trn_type="TRN2")
    x_d = nc.dram_tensor("x", [B_PER_, H_, W_], f32, kind="ExternalInput")
    y_d = nc.dram_tensor("y", [B_PER_, H_, W_], f16, kind="ExternalOutput")

    consts = {}
    for m in sorted({STRIDE_, H_ - STRIDE_ * (N_STRIPS_ - 1)}):
        m0_np, m1_np = _band_matrices(m)
        consts[m] = (
            nc.inline_tensor(m0_np, f"m0_const_{m}"),
            nc.inline_tensor(m1_np, f"m1_const_{m}"),
        )

    with TileContext(nc) as tc:
        with (
            tc.tile_pool(name="wpool", bufs=1) as wpool,
            tc.tile_pool(name="xpool", bufs=6) as xpool,
            tc.tile_pool(name="apool", bufs=4) as apool,
            tc.tile_pool(name="spool", bufs=4) as spool,
            tc.tile_pool(name="tpool", bufs=4) as tpool,
            tc.tile_pool(name="opool", bufs=6) as opool,
            tc.tile_pool(name="ppool", bufs=2, space="PSUM") as ppool,
        ):
            bands = {}
            for m, (m0_d, m1_d) in consts.items():
                m0 = wpool.tile([m + 2, m], f16, name=f"m0_{m}")
                m1 = wpool.tile([m + 2, m], f16, name=f"m1_{m}")
                nc.sync.dma_start(out=m0[:], in_=m0_d[:])
                nc.sync.dma_start(out=m1[:], in_=m1_d[:])
                bands[m] = (m0, m1)

            # activation biases must be [128,1] APs, not immediates
            bm25 = wpool.tile([128, 1], f32)
            bp10 = wpool.tile([128, 1], f32)
            bm2p5 = wpool.tile([128, 1], f32)
            nc.vector.memset(bm25[:], -25.0)
            nc.vector.memset(bp10[:], 10.0)
            nc.vector.memset(bm2p5[:], -2.5)

            for b in range(B_PER_):
                for t in range(N_STRIPS_):
                    r0 = t * STRIDE_
                    M = min(STRIDE_, H_ - r0)  # output rows this strip
                    k = M + 2
                    m0, m1 = bands[M]

                    # fp16 tile, partitions 0..M-1 = cells (rows r0..),
                    # partition M = bottom halo, M+1 = top halo.  gpsimd
                    # (SWDGE) DMA casts fp32->fp16 in flight.
                    xt = xpool.tile([KROWS, W_], f16, tag="xt")
                    if r0 + M < H_:
                        nc.gpsimd.dma_start(
                            out=xt[0 : M + 1, :], in_=x_d[b, r0 : r0 + M + 1, :]
                        )
                    else:
                        # last strip: bottom halo wraps to row 0
                        nc.gpsimd.dma_start(out=xt[0:M, :], in_=x_d[b, r0:H_, :])
                        nc.gpsimd.dma_start(out=xt[M : M + 1, :], in_=x_d[b, 0:1, :])
                    rtop = (r0 - 1) % H_
                    nc.gpsimd.dma_start(
                        out=xt[M + 1 : M + 2, :], in_=x_d[b, rtop : rtop + 1, :]
                    )

                    ps = ppool.tile([STRIDE_, W_], f32, tag="ps")
                    m0s = m0[:k, :M]
                    m1s = m1[:k, :M]

                    # Pre-touch: a 1x1 matmul absorbs the PSUM-release wait
                    # (Matmult carries at most ONE sync wait; without this,
                    # wait-merging couples strip t to strip t-1's consumers
                    # and serializes PE behind ACT/DVE).
                    nc.tensor.matmul(
                        ps[:1, 0:1], bm25[:1, :1], bm25[:1, :1],
                        start=True, stop=True,
                    )

                    # around = sum of 8 neighbors, accumulated in PSUM.
                    for nb in range(NBANKS_):
                        c0 = nb * 512
                        c1 = c0 + 512
                        # center column, vertical neighbors only
                        nc.tensor.matmul(
                            ps[:M, c0:c1], m0s, xt[:k, c0:c1],
                            start=True, stop=False,
                        )
                        # left-neighbor column: out col j += band @ x col j-1
                        if nb == 0:
                            nc.tensor.matmul(
                                ps[:M, 1:512], m1s, xt[:k, 0:511],
                                start=False, stop=False,
                            )
                            nc.tensor.matmul(
                                ps[:M, 0:1], m1s, xt[:k, W_ - 1 : W_],
                                start=False, stop=False,
                            )
                        else:
                            nc.tensor.matmul(
                                ps[:M, c0:c1], m1s, xt[:k, c0 - 1 : c1 - 1],
                                start=False, stop=False,
                            )
                        # right-neighbor column: out col j += band @ x col j+1
                        if nb == NBANKS_ - 1:
                            nc.tensor.matmul(
                                ps[:M, c0 : W_ - 1], m1s, xt[:k, c0 + 1 : W_],
                                start=False, stop=False,
                            )
                            nc.tensor.matmul(
                                ps[:M, W_ - 1 : W_], m1s, xt[:k, 0:1],
                                start=False, stop=True,
                            )
                        else:
                            nc.tensor.matmul(
                                ps[:M, c0:c1], m1s, xt[:k, c0 + 1 : c1 + 1],
                                start=False, stop=True,
                            )

                    # w = |around - 2.5|, split ScalarE / VectorE
                    wt = apool.tile([STRIDE_, W_], f16, tag="wt")
                    nc.scalar.activation(
                        wt[:M, 0:ABS_ACT_W], ps[:M, 0:ABS_ACT_W], AbsF,
                        bias=bm2p5[:M], scale=1.0,
                    )
                    nc.vector._custom_dve(
                        absshift, out=wt[:M, ABS_ACT_W:], in0=ps[:M, ABS_ACT_W:],
                        s0=-2.5,
                    )

                    # E2 = sigmoid(10*around - 25); E1 = sigmoid(10 - 10*w)
                    e2 = spool.tile([STRIDE_, W_], f16, tag="e2")
                    e1 = spool.tile([STRIDE_, W_], f16, tag="e1")
                    nc.scalar.activation(e2[:M], ps[:M], Sig, bias=bm25[:M], scale=10.0)
                    nc.scalar.activation(e1[:M], wt[:M], Sig, bias=bp10[:M], scale=-10.0)

                    # t = x + E2 - x*E2  (fused custom DVE op)
                    tt = tpool.tile([STRIDE_, W_], f16, tag="tt")
                    if USE_CUSTOM_BLEND:
                        bi = nc.vector._custom_dve(
                            blend1, out=tt[:M], in0=xt[:M, :], in1=e2[:M]
                        )
                        bi.ins.perf_max = 1
                    else:
                        mm = tpool.tile([STRIDE_, W_], f16, tag="mm")
                        nc.vector.tensor_mul(out=mm[:M], in0=xt[:M, :], in1=e2[:M])
                        nc.vector.tensor_sub(out=mm[:M], in0=e2[:M], in1=mm[:M])
                        nc.vector.tensor_add(out=tt[:M], in0=xt[:M, :], in1=mm[:M])

                    # out = E1 * t, split VectorE / GpSimd
                    o = opool.tile([STRIDE_, W_], f16, tag="o")
                    nc.vector.tensor_mul(
                        out=o[:M, 0:MUL_DVE_W], in0=e1[:M, 0:MUL_DVE_W],
                        in1=tt[:M, 0:MUL_DVE_W],
                    )
                    nc.gpsimd.tensor_mul(
                        out=o[:M, MUL_DVE_W:], in0=e1[:M, MUL_DVE_W:],
                        in1=tt[:M, MUL_DVE_W:],
                    )
                    nc.sync.dma_start(out=y_d[b, r0 : r0 + M, :], in_=o[:M])

    nc.compile()
    if (b_per, h, w, stride) == (B_PER, H, W, STRIDE):
        _cached_nc = nc
    return nc


def run(x, trace=False):
    """Run the SPMD kernel on 8 cores. Returns (out_fp32, BassKernelResults)."""
    from concourse.bass_utils import run_bass_kernel_spmd

    nc = _build()
    x = np.asarray(x, dtype=np.float32)
    assert x.shape == (B, H, W), x.shape
    in_maps = [{"x": x[B_PER * c : B_PER * (c + 1)]} for c in range(N_CORES)]
    res = run_bass_kernel_spmd(nc, in_maps, core_ids=list(range(N_CORES)), trace=trace)
    out = np.concatenate(
        [res.results[c]["y"].astype(np.float32) for c in range(N_CORES)], axis=0
    )
    return out, res


def kernel(x):
    out, _ = run(x, trace=False)
    return out


# revision 13
# speedup vs baseline: 1.0102x; 1.0102x over previous
"""Continuous Game-of-Life Trainium2 kernel (product-form, 2-sigmoid).

Reference computation (per batch image, cyclic 3x3 stencil):
    around = 8-neighbor sum of x (torus wrap), u = 10*around
    survive = sigmoid(u-15) * sigmoid(35-u)
    birth   = sigmoid(u-25) * sigmoid(35-u)
    out     = x*survive + (1-x)*birth

Math used here (max abs err ~5e-5 vs reference, fp64):
    E1 := sigmoid(10 - |u-25|)        # == survive (err <= sigmoid(-10))
    E2 := sigmoid(u-25)
    birth == E1*E2 (err ~5e-5), so
    out = E1 * (x + E2 - x*E2)

This needs only TWO sigmoid passes on the Scalar engine (the baseline
three-sigmoid form is ScalarE-bound at ~196us busy).  The remaining
work is spread to keep every engine under the ~4.4us/strip DMA floor:
  - TensorE: 8-neighbor sum via banded matmuls (as before).
  - abs pass w = |around-2.5|: split ScalarE (Abs activation, ~30%) /
    VectorE (tensor_scalar add+abs_max, ~70%; PSUM source runs 1x).
  - blend t = x + E2 - x*E2: one fused custom-DVE op (BLEND1_ANT).
  - out = E1*t: split VectorE (2x fp16) / GpSimd.
  - DMA in fp32->fp16 (SWDGE cast), out fp16.

Sharding: data-parallel over batch: 16 images -> 8 cores x 2 images.
Torus wrap is per-image so there is no cross-core halo.
"""

import numpy as np

B, H, W = 16, 2048, 2048
N_CORES = 8
B_PER = B // N_CORES  # 2 images per core
STRIDE = 126  # output rows per strip (128 input rows incl. halos)
N_STRIPS = (H + STRIDE - 1) // STRIDE  # 17
NBANKS = W // 512  # PSUM banks per strip

# work-split knobs (elements of the 2048-wide free dim)
ABS_ACT_W = 320  # abs columns done on ScalarE (rest on VectorE)
MUL_DVE_W = 1024  # final-mul columns done on VectorE (rest on GpSimd)
USE_CUSTOM_BLEND = True

_cached_nc = None
_custom_ops = None


def _register_custom_ops():
    """Register fused custom DVE ops at runtime.

    Same mechanism as editing dve_ops.py (the per-NEFF uop table is
    generated at compile time from OPS); the sha is computed here so the
    pin always matches this interpreter's lowering.

      BLEND1_ANT:    out = in0 + in1 - in0*in1
      ABS_SHIFT_ANT: out = |in0 + s0|   (walrus rejects abs_max on
                     TensorScalar, so plain TS cannot do an abs)
    """
    global _custom_ops
    if _custom_ops is not None:
        return _custom_ops
    import numpy as np

    from concourse import dve_ops
    from concourse.dve_spec import C0, Spec, Src0, Src1, Zero, lower, maxx
    from concourse.dve_uop import DveOpSpec

    def _mk(name, spec):
        if name in dve_ops._SUB_OPCODE_FOR_NAME:
            return next(op for op in dve_ops.OPS if op.name == name)
        shas = {
            ver: DveOpSpec(
                name=name, opcode=0, uops=lower(spec, ver=ver), rd1_en=True
            ).sha(ver)
            for ver in ("v3", "v4")
        }
        op = dve_ops.DveOp(name, spec, subdim=False, uops_sha=shas)
        row = dve_ops._CUSTOM_DVE_ROW_BASE + len(dve_ops.OPS)
        assert row < 0x20
        dve_ops.OPS.append(op)
        dve_ops._SUB_OPCODE_FOR_NAME[name] = row
        dve_ops.CUSTOM_DVE_SPECS[name] = spec
        return op

    blend = _mk(
        "BLEND1_ANT",
        Spec(
            body=Src0 + Src1 - Src0 * Src1,
            reference=lambda in0, in1, s0, s1, imm2: in0 + in1 - in0 * in1,
        ),
    )
    _y = Src0 + C0
    absshift = _mk(
        "ABS_SHIFT_ANT",
        Spec(
            body=maxx(_y, Zero - _y),
            reference=lambda in0, in1, s0, s1, imm2: np.abs(in0 + s0),
        ),
    )
    _attach_blend_2x(blend)
    _custom_ops = (blend, absshift)
    return _custom_ops


def _attach_blend_2x(op):
    """Hand-authored 2x_1p uop program for BLEND1_ANT.

    lower() only emits the 1x program; the engine's perf-mode slots are
    real hardware (control_table[table_ptr+mode]) and dve_table_gen writes
    them when DveOpSpec.uops_2x is set.  Program (stock TENSOR_TENSOR
    2x_1p pattern, hi result carried on delay lane 1):
      entry: alu-slot=a_lo, d0=b_lo, d1=a_hi, d2=b_hi
      S0 s_hi=a_hi+b_hi (a_lo->d3)   S3 s_lo=a_lo+b_lo (t_hi->d1)
      S1 m_hi=a_hi*b_hi (s_hi->d4)   S4 m_lo=a_lo*b_lo (s_lo->d2)
      S2 t_hi=s_hi-m_hi              S5 t_lo=s_lo-m_lo
      write: WR0_LO<-ALU_OUT (t_lo), WR0_HI<-DELAY_1 (t_hi)
    Verified bit-exact vs the 1x program on hardware (test_blend2x.py).
    """
    from concourse import dve_ops
    from concourse.dve_spec import lower
    from concourse.dve_uop import (
        AluInp as A,
        AluOp as U,
        DelayInp,
        DveOpSpec,
        InpSel,
        OutPath,
        OutSel,
        Trigger,
        UopConfig,
        UopDpConfig,
    )

    PD = DelayInp.PREV_DELAY
    PAO = DelayInp.PREV_ALU_OUT

    def dp(opc, s0, s1, delays):
        d = [PD] * 7
        de = [0] * 7
        for lane, src in delays.items():
            d[lane] = src
            de[lane] = 1
        return UopDpConfig(
            op=opc, alu_src0=s0, alu_src1=s1, delay=d, delay_enable=de,
            alu_out_enable=1,
        )

    stages = [
        dp(U.ADD, A.PREV_DELAY_1, A.PREV_DELAY_2, {0: PD, 1: PD, 2: PD, 3: PAO}),
        dp(U.MULTIPLY, A.PREV_DELAY_1, A.PREV_DELAY_2, {0: PD, 3: PD, 4: PAO}),
        dp(U.SUBTRACT, A.PREV_DELAY_4, A.PREV_ALU_OUT, {0: PD, 3: PD}),
        dp(U.ADD, A.PREV_DELAY_3, A.PREV_DELAY_0, {0: PD, 1: PAO, 3: PD}),
        dp(U.MULTIPLY, A.PREV_DELAY_3, A.PREV_DELAY_0, {1: PD, 2: PAO}),
        dp(U.SUBTRACT, A.PREV_DELAY_2, A.PREV_ALU_OUT, {1: PD}),
        dp(U.BYPASS, A.PREV_ALU_OUT, A.PREV_ALU_OUT, {1: PD}),
        dp(U.BYPASS, A.PREV_ALU_OUT, A.PREV_ALU_OUT, {1: PD}),
    ]
    u2 = UopConfig(
        inp=[InpSel.SRC_0, InpSel.SRC_1, InpSel.SRC_0_HI, InpSel.SRC_1_HI]
        + [InpSel.ZERO] * 4,
        inp_enable=[1, 1, 1, 1, 0, 0, 0, 0],
        out={
            OutPath.WR0_LO: OutSel.ALU_OUT,
            OutPath.WR0_HI: OutSel.DELAY_1,
            OutPath.WR1_LO: OutSel.ALU_OUT,
            OutPath.WR1_HI: OutSel.ALU_OUT,
        },
        out_enable={
            OutPath.WR0_LO: 1, OutPath.WR0_HI: 1, OutPath.WR1_LO: 0, OutPath.WR1_HI: 0,
        },
        require_inp0=1,
        require_inp1=1,
        trigger=(Trigger.SRC_TENSOR_DONE, Trigger.NONE, Trigger.NONE),
        next_uop=(0, 0, 0),
        datapath_config=stages,
    )
    ver = "v3"
    u2.validate(ver)
    spec = DveOpSpec(
        name=op.name,
        opcode=dve_ops.get_dve_sub_opcode(op.name),
        uops=lower(op.spec, ver=ver),
        uops_2x=[u2],
        rd1_en=True,
        perf_max=1,
    )
    spec.validate(ver)
    dve_ops._COMPILE_CACHE[(op.name, ver)] = spec


def _band_matrices(m, dtype=np.float16):
    """[m+2, m] stationary operands for the vertical taps.

    Tile layout: partitions 0..m-1 hold image rows r0..r0+m-1 (the cells),
    partition m holds the bottom halo row r0+m, partition m+1 holds the top
    halo row r0-1.  For output row p the vertical neighbors are partitions
    p-1 (or m+1 when p==0) and p+1.

    m0[k, p] = 1 for the two vertical neighbors (no center),
    m1[k, p] = 1 for the full 3-tap (used on the column-shifted views).
    """
    m0 = np.zeros((m + 2, m), dtype)
    m1 = np.zeros((m + 2, m), dtype)
    for p in range(m):
        up = m + 1 if p == 0 else p - 1
        m0[up, p] = 1.0
        m0[p + 1, p] = 1.0
        m1[up, p] = 1.0
        m1[p, p] = 1.0
        m1[p + 1, p] = 1.0
    return m0, m1


def _build(b_per=B_PER, h=H, w=W, stride=STRIDE):
    global _cached_nc
    if _cached_nc is not None and (b_per, h, w, stride) == (B_PER, H, W, STRIDE):
        return _cached_nc

    import concourse.mybir as mybir
    from concourse.bacc import Bacc
    from concourse.tile import TileContext

    blend1, absshift = _register_custom_ops()

    B_PER_, H_, W_, STRIDE_ = b_per, h, w, stride
    N_STRIPS_ = (H_ + STRIDE_ - 1) // STRIDE_
    NBANKS_ = W_ // 512
    KROWS = STRIDE_ + 2

    f32 = mybir.dt.float32
    f16 = mybir.dt.float16
    Sig = mybir.ActivationFunctionType.Sigmoid
    AbsF = mybir.ActivationFunctionType.Abs
    Add = mybir.AluOpType.add
    AbsMax = mybir.AluOpType.abs_max

    nc = Bacc(trn_type="TRN2")
    x_d = nc.dram_tensor("x", [B_PER_, H_, W_], f32, kind="ExternalInput")
    y_d = nc.dram_tensor("y", [B_PER_, H_, W_], f16, kind="ExternalOutput")

    consts = {}
    for m in sorted({STRIDE_, H_ - STRIDE_ * (N_STRIPS_ - 1)}):
        m0_np, m1_np = _band_matrices(m)
        consts[m] = (
            nc.inline_tensor(m0_np, f"m0_const_{m}"),
            nc.inline_tensor(m1_np, f"m1_const_{m}"),
        )

    with TileContext(nc) as tc:
        with (
            tc.tile_pool(name="wpool", bufs=1) as wpool,
            tc.tile_pool(name="xpool", bufs=6) as xpool,
            tc.tile_pool(name="apool", bufs=4) as apool,
            tc.tile_pool(name="spool", bufs=4) as spool,
            tc.tile_pool(name="tpool", bufs=4) as tpool,
            tc.tile_pool(name="opool", bufs=6) as opool,
            tc.tile_pool(name="ppool", bufs=2, space="PSUM") as ppool,
        ):
            bands = {}
            for m, (m0_d, m1_d) in consts.items():
                m0 = wpool.tile([m + 2, m], f16, name=f"m0_{m}")
                m1 = wpool.tile([m + 2, m], f16, name=f"m1_{m}")
                nc.sync.dma_start(out=m0[:], in_=m0_d[:])
                nc.sync.dma_start(out=m1[:], in_=m1_d[:])
                bands[m] = (m0, m1)

            # activation biases must be [128,1] APs, not immediates
            bm25 = wpool.tile([128, 1], f32)
            bp10 = wpool.tile([128, 1], f32)
            bm2p5 = wpool.tile([128, 1], f32)
            nc.vector.memset(bm25[:], -25.0)
            nc.vector.memset(bp10[:], 10.0)
            nc.vector.memset(bm2p5[:], -2.5)

            strips = [(b, t) for b in range(B_PER_) for t in range(N_STRIPS_)]
            xt_tiles = {}

            def issue_load(b, t):
                """fp16 tile, partitions 0..M-1 = cells (rows r0..),
                partition M = bottom halo, M+1 = top halo.  gpsimd
                (SWDGE) DMA casts fp32->fp16 in flight."""
                r0 = t * STRIDE_
                M = min(STRIDE_, H_ - r0)
                xt = xpool.tile([KROWS, W_], f16, tag="xt")
                if r0 + M < H_:
                    nc.gpsimd.dma_start(
                        out=xt[0 : M + 1, :], in_=x_d[b, r0 : r0 + M + 1, :]
                    )
                else:
                    # last strip: bottom halo wraps to row 0
                    nc.gpsimd.dma_start(out=xt[0:M, :], in_=x_d[b, r0:H_, :])
                    nc.gpsimd.dma_start(out=xt[M : M + 1, :], in_=x_d[b, 0:1, :])
                rtop = (r0 - 1) % H_
                nc.gpsimd.dma_start(
                    out=xt[M + 1 : M + 2, :], in_=x_d[b, rtop : rtop + 1, :]
                )
                xt_tiles[(b, t)] = xt

            # Input DMA issued LOOKAHEAD strips ahead of compute so the Pool
            # sequencer's per-strip mul never gates the next strip's load
            # (the SWDGE trigger and the gpsimd mul share the Pool queue).
            LOOKAHEAD = 3
            next_load = 0

            for i, (b, t) in enumerate(strips):
                    while next_load < min(i + LOOKAHEAD, len(strips)):
                        issue_load(*strips[next_load])
                        next_load += 1
                    r0 = t * STRIDE_
                    M = min(STRIDE_, H_ - r0)  # output rows this strip
                    k = M + 2
                    m0, m1 = bands[M]
                    xt = xt_tiles.pop((b, t))

                    ps = ppool.tile([STRIDE_, W_], f32, tag="ps")
                    m0s = m0[:k, :M]
                    m1s = m1[:k, :M]

                    # Pre-touch: a 1x1 matmul absorbs the PSUM-release wait
                    # (Matmult carries at most ONE sync wait; without this,
                    # wait-merging couples strip t to strip t-1's consumers
                    # and serializes PE behind ACT/DVE).
                    nc.tensor.matmul(
                        ps[:1, 0:1], bm25[:1, :1], bm25[:1, :1],
                        start=True, stop=True,
                    )

                    # around = sum of 8 neighbors, accumulated in PSUM.
                    for nb in range(NBANKS_):
                        c0 = nb * 512
                        c1 = c0 + 512
                        # center column, vertical neighbors only
                        nc.tensor.matmul(
                            ps[:M, c0:c1], m0s, xt[:k, c0:c1],
                            start=True, stop=False,
                        )
                        # left-neighbor column: out col j += band @ x col j-1
                        if nb == 0:
                            nc.tensor.matmul(
                                ps[:M, 1:512], m1s, xt[:k, 0:511],
                                start=False, stop=False,
                            )
                            nc.tensor.matmul(
                                ps[:M, 0:1], m1s, xt[:k, W_ - 1 : W_],
                                start=False, stop=False,
                            )
                        else:
                            nc.tensor.matmul(
                                ps[:M, c0:c1], m1s, xt[:k, c0 - 1 : c1 - 1],
                                start=False, stop=False,
                            )
                        # right-neighbor column: out col j += band @ x col j+1
                        if nb == NBANKS_ - 1:
                            nc.tensor.matmul(
                                ps[:M, c0 : W_ - 1], m1s, xt[:k, c0 + 1 : W_],
                                start=False, stop=False,
                            )
                            nc.tensor.matmul(
                                ps[:M, W_ - 1 : W_], m1s, xt[:k, 0:1],
                                start=False, stop=True,
                            )
                        else:
                            nc.tensor.matmul(
                                ps[:M, c0:c1], m1s, xt[:k, c0 + 1 : c1 + 1],
                                start=False, stop=True,
                            )

                    # w = |around - 2.5|, split ScalarE / VectorE
                    wt = apool.tile([STRIDE_, W_], f16, tag="wt")
                    nc.scalar.activation(
                        wt[:M, 0:ABS_ACT_W], ps[:M, 0:ABS_ACT_W], AbsF,
                        bias=bm2p5[:M], scale=1.0,
                    )
                    nc.vector._custom_dve(
                        absshift, out=wt[:M, ABS_ACT_W:], in0=ps[:M, ABS_ACT_W:],
                        s0=-2.5,
                    )

                    # E2 = sigmoid(10*around - 25); E1 = sigmoid(10 - 10*w)
                    e2 = spool.tile([STRIDE_, W_], f16, tag="e2")
                    e1 = spool.tile([STRIDE_, W_], f16, tag="e1")
                    nc.scalar.activation(e2[:M], ps[:M], Sig, bias=bm25[:M], scale=10.0)
                    nc.scalar.activation(e1[:M], wt[:M], Sig, bias=bp10[:M], scale=-10.0)

                    # t = x + E2 - x*E2  (fused custom DVE op)
                    tt = tpool.tile([STRIDE_, W_], f16, tag="tt")
                    if USE_CUSTOM_BLEND:
                        bi = nc.vector._custom_dve(
                            blend1, out=tt[:M], in0=xt[:M, :], in1=e2[:M]
                        )
                        bi.ins.perf_max = 1
                    else:
                        mm = tpool.tile([STRIDE_, W_], f16, tag="mm")
                        nc.vector.tensor_mul(out=mm[:M], in0=xt[:M, :], in1=e2[:M])
                        nc.vector.tensor_sub(out=mm[:M], in0=e2[:M], in1=mm[:M])
                        nc.vector.tensor_add(out=tt[:M], in0=xt[:M, :], in1=mm[:M])

                    # out = E1 * t, split VectorE / GpSimd
                    o = opool.tile([STRIDE_, W_], f16, tag="o")
                    nc.vector.tensor_mul(
                        out=o[:M, 0:MUL_DVE_W], in0=e1[:M, 0:MUL_DVE_W],
                        in1=tt[:M, 0:MUL_DVE_W],
                    )
                    nc.gpsimd.tensor_mul(
                        out=o[:M, MUL_DVE_W:], in0=e1[:M, MUL_DVE_W:],
                        in1=tt[:M, MUL_DVE_W:],
                    )
                    nc.sync.dma_start(out=y_d[b, r0 : r0 + M, :], in_=o[:M])

    nc.compile()
    if (b_per, h, w, stride) == (B_PER, H, W, STRIDE):
        _cached_nc = nc
    return nc


def run(x, trace=False):
    """Run the SPMD kernel on 8 cores. Returns (out_fp32, BassKernelResults)."""
    from concourse.bass_utils import run_bass_kernel_spmd

    nc = _build()
    x = np.asarray(x, dtype=np.float32)
    assert x.shape == (B, H, W), x.shape
    in_maps = [{"x": x[B_PER * c : B_PER * (c + 1)]} for c in range(N_CORES)]
    res = run_bass_kernel_spmd(nc, in_maps, core_ids=list(range(N_CORES)), trace=trace)
    out = np.concatenate(
        [res.results[c]["y"].astype(np.float32) for c in range(N_CORES)], axis=0
    )
    return out, res


def kernel(x):
    out, _ = run(x, trace=False)
    return out


# revision 17
# speedup vs baseline: 1.0222x; 1.0119x over previous
"""Continuous Game-of-Life Trainium2 kernel (product-form, 2-sigmoid).

Reference computation (per batch image, cyclic 3x3 stencil):
    around = 8-neighbor sum of x (torus wrap), u = 10*around
    survive = sigmoid(u-15) * sigmoid(35-u)
    birth   = sigmoid(u-25) * sigmoid(35-u)
    out     = x*survive + (1-x)*birth

Math used here (max abs err ~5e-5 vs reference, fp64):
    E1 := sigmoid(10 - |u-25|)        # == survive (err <= sigmoid(-10))
    E2 := sigmoid(u-25)
    birth == E1*E2 (err ~5e-5), so
    out = E1 * (x + E2 - x*E2)

This needs only TWO sigmoid passes on the Scalar engine (the baseline
three-sigmoid form is ScalarE-bound at ~196us busy).  The remaining
work is spread to keep every engine under the ~4.4us/strip DMA floor:
  - TensorE: 8-neighbor sum via banded matmuls (as before).
  - abs pass w = |around-2.5|: split ScalarE (Abs activation, ~30%) /
    VectorE (tensor_scalar add+abs_max, ~70%; PSUM source runs 1x).
  - blend t = x + E2 - x*E2: one fused custom-DVE op (BLEND1_ANT).
  - out = E1*t: split VectorE (2x fp16) / GpSimd.
  - DMA in fp32->fp16 (SWDGE cast), out fp16.

Sharding: data-parallel over batch: 16 images -> 8 cores x 2 images.
Torus wrap is per-image so there is no cross-core halo.
"""

import numpy as np

B, H, W = 16, 2048, 2048
N_CORES = 8
B_PER = B // N_CORES  # 2 images per core
STRIDE = 126  # output rows per strip (128 input rows incl. halos)
N_STRIPS = (H + STRIDE - 1) // STRIDE  # 17
NBANKS = W // 512  # PSUM banks per strip

# work-split knobs (elements of the 2048-wide free dim)
ABS_ACT_W = 256  # abs columns done on ScalarE (rest on VectorE)
MUL_DVE_W = 1024  # final-mul columns done on VectorE (rest on GpSimd)
USE_CUSTOM_BLEND = True

_cached_nc = None
_custom_ops = None


def _register_custom_ops():
    """Register fused custom DVE ops at runtime.

    Same mechanism as editing dve_ops.py (the per-NEFF uop table is
    generated at compile time from OPS); the sha is computed here so the
    pin always matches this interpreter's lowering.

      BLEND1_ANT:    out = in0 + in1 - in0*in1
      ABS_SHIFT_ANT: out = |in0 + s0|   (walrus rejects abs_max on
                     TensorScalar, so plain TS cannot do an abs)
    """
    global _custom_ops
    if _custom_ops is not None:
        return _custom_ops
    import numpy as np

    from concourse import dve_ops
    from concourse.dve_spec import C0, Spec, Src0, Src1, Zero, lower, maxx
    from concourse.dve_uop import DveOpSpec

    def _mk(name, spec):
        if name in dve_ops._SUB_OPCODE_FOR_NAME:
            return next(op for op in dve_ops.OPS if op.name == name)
        shas = {
            ver: DveOpSpec(
                name=name, opcode=0, uops=lower(spec, ver=ver), rd1_en=True
            ).sha(ver)
            for ver in ("v3", "v4")
        }
        op = dve_ops.DveOp(name, spec, subdim=False, uops_sha=shas)
        row = dve_ops._CUSTOM_DVE_ROW_BASE + len(dve_ops.OPS)
        assert row < 0x20
        dve_ops.OPS.append(op)
        dve_ops._SUB_OPCODE_FOR_NAME[name] = row
        dve_ops.CUSTOM_DVE_SPECS[name] = spec
        return op

    blend = _mk(
        "BLEND1_ANT",
        Spec(
            body=Src0 + Src1 - Src0 * Src1,
            reference=lambda in0, in1, s0, s1, imm2: in0 + in1 - in0 * in1,
        ),
    )
    _y = Src0 + C0
    absshift = _mk(
        "ABS_SHIFT_ANT",
        Spec(
            body=maxx(_y, Zero - _y),
            reference=lambda in0, in1, s0, s1, imm2: np.abs(in0 + s0),
        ),
    )
    _attach_blend_2x(blend)
    _custom_ops = (blend, absshift)
    return _custom_ops


def _attach_blend_2x(op):
    """Hand-authored 2x_1p uop program for BLEND1_ANT.

    lower() only emits the 1x program; the engine's perf-mode slots are
    real hardware (control_table[table_ptr+mode]) and dve_table_gen writes
    them when DveOpSpec.uops_2x is set.  Program (stock TENSOR_TENSOR
    2x_1p pattern, hi result carried on delay lane 1):
      entry: alu-slot=a_lo, d0=b_lo, d1=a_hi, d2=b_hi
      S0 s_hi=a_hi+b_hi (a_lo->d3)   S3 s_lo=a_lo+b_lo (t_hi->d1)
      S1 m_hi=a_hi*b_hi (s_hi->d4)   S4 m_lo=a_lo*b_lo (s_lo->d2)
      S2 t_hi=s_hi-m_hi              S5 t_lo=s_lo-m_lo
      write: WR0_LO<-ALU_OUT (t_lo), WR0_HI<-DELAY_1 (t_hi)
    Verified bit-exact vs the 1x program on hardware (test_blend2x.py).
    """
    from concourse import dve_ops
    from concourse.dve_spec import lower
    from concourse.dve_uop import (
        AluInp as A,
        AluOp as U,
        DelayInp,
        DveOpSpec,
        InpSel,
        OutPath,
        OutSel,
        Trigger,
        UopConfig,
        UopDpConfig,
    )

    PD = DelayInp.PREV_DELAY
    PAO = DelayInp.PREV_ALU_OUT

    def dp(opc, s0, s1, delays):
        d = [PD] * 7
        de = [0] * 7
        for lane, src in delays.items():
            d[lane] = src
            de[lane] = 1
        return UopDpConfig(
            op=opc, alu_src0=s0, alu_src1=s1, delay=d, delay_enable=de,
            alu_out_enable=1,
        )

    stages = [
        dp(U.ADD, A.PREV_DELAY_1, A.PREV_DELAY_2, {0: PD, 1: PD, 2: PD, 3: PAO}),
        dp(U.MULTIPLY, A.PREV_DELAY_1, A.PREV_DELAY_2, {0: PD, 3: PD, 4: PAO}),
        dp(U.SUBTRACT, A.PREV_DELAY_4, A.PREV_ALU_OUT, {0: PD, 3: PD}),
        dp(U.ADD, A.PREV_DELAY_3, A.PREV_DELAY_0, {0: PD, 1: PAO, 3: PD}),
        dp(U.MULTIPLY, A.PREV_DELAY_3, A.PREV_DELAY_0, {1: PD, 2: PAO}),
        dp(U.SUBTRACT, A.PREV_DELAY_2, A.PREV_ALU_OUT, {1: PD}),
        dp(U.BYPASS, A.PREV_ALU_OUT, A.PREV_ALU_OUT, {1: PD}),
        dp(U.BYPASS, A.PREV_ALU_OUT, A.PREV_ALU_OUT, {1: PD}),
    ]
    u2 = UopConfig(
        inp=[InpSel.SRC_0, InpSel.SRC_1, InpSel.SRC_0_HI, InpSel.SRC_1_HI]
        + [InpSel.ZERO] * 4,
        inp_enable=[1, 1, 1, 1, 0, 0, 0, 0],
        out={
            OutPath.WR0_LO: OutSel.ALU_OUT,
            OutPath.WR0_HI: OutSel.DELAY_1,
            OutPath.WR1_LO: OutSel.ALU_OUT,
            OutPath.WR1_HI: OutSel.ALU_OUT,
        },
        out_enable={
            OutPath.WR0_LO: 1, OutPath.WR0_HI: 1, OutPath.WR1_LO: 0, OutPath.WR1_HI: 0,
        },
        require_inp0=1,
        require_inp1=1,
        trigger=(Trigger.SRC_TENSOR_DONE, Trigger.NONE, Trigger.NONE),
        next_uop=(0, 0, 0),
        datapath_config=stages,
    )
    ver = "v3"
    u2.validate(ver)
    spec = DveOpSpec(
        name=op.name,
        opcode=dve_ops.get_dve_sub_opcode(op.name),
        uops=lower(op.spec, ver=ver),
        uops_2x=[u2],
        rd1_en=True,
        perf_max=1,
    )
    spec.validate(ver)
    dve_ops._COMPILE_CACHE[(op.name, ver)] = spec


def _band_matrices(m, dtype=np.float16):
    """[m+2, m] stationary operands for the vertical taps.

    Tile layout: partitions 0..m-1 hold image rows r0..r0+m-1 (the cells),
    partition m holds the bottom halo row r0+m, partition m+1 holds the top
    halo row r0-1.  For output row p the vertical neighbors are partitions
    p-1 (or m+1 when p==0) and p+1.

    m0[k, p] = 1 for the two vertical neighbors (no center),
    m1[k, p] = 1 for the full 3-tap (used on the column-shifted views).
    """
    m0 = np.zeros((m + 2, m), dtype)
    m1 = np.zeros((m + 2, m), dtype)
    for p in range(m):
        up = m + 1 if p == 0 else p - 1
        m0[up, p] = 1.0
        m0[p + 1, p] = 1.0
        m1[up, p] = 1.0
        m1[p, p] = 1.0
        m1[p + 1, p] = 1.0
    return m0, m1


def _build(b_per=B_PER, h=H, w=W, stride=STRIDE):
    global _cached_nc
    if _cached_nc is not None and (b_per, h, w, stride) == (B_PER, H, W, STRIDE):
        return _cached_nc

    import concourse.mybir as mybir
    from concourse.bacc import Bacc
    from concourse.tile import TileContext

    blend1, absshift = _register_custom_ops()

    B_PER_, H_, W_, STRIDE_ = b_per, h, w, stride
    N_STRIPS_ = (H_ + STRIDE_ - 1) // STRIDE_
    NBANKS_ = W_ // 512
    KROWS = STRIDE_ + 2

    f32 = mybir.dt.float32
    f16 = mybir.dt.float16
    Sig = mybir.ActivationFunctionType.Sigmoid
    AbsF = mybir.ActivationFunctionType.Abs

    nc = Bacc(trn_type="TRN2")
    x_d = nc.dram_tensor("x", [B_PER_, H_, W_], f32, kind="ExternalInput")
    y_d = nc.dram_tensor("y", [B_PER_, H_, W_], f16, kind="ExternalOutput")

    consts = {}
    for m in sorted({STRIDE_, H_ - STRIDE_ * (N_STRIPS_ - 1)}):
        m0_np, m1_np = _band_matrices(m)
        consts[m] = (
            nc.inline_tensor(m0_np, f"m0_const_{m}"),
            nc.inline_tensor(m1_np, f"m1_const_{m}"),
        )

    with TileContext(nc) as tc:
        with (
            tc.tile_pool(name="wpool", bufs=1) as wpool,
            tc.tile_pool(name="xpool", bufs=8) as xpool,
            tc.tile_pool(name="apool", bufs=4) as apool,
            tc.tile_pool(name="spool", bufs=4) as spool,
            tc.tile_pool(name="tpool", bufs=4) as tpool,
            tc.tile_pool(name="opool", bufs=6) as opool,
            tc.tile_pool(name="ppool", bufs=2, space="PSUM") as ppool,
        ):
            bands = {}
            for m, (m0_d, m1_d) in consts.items():
                m0 = wpool.tile([m + 2, m], f16, name=f"m0_{m}")
                m1 = wpool.tile([m + 2, m], f16, name=f"m1_{m}")
                nc.sync.dma_start(out=m0[:], in_=m0_d[:])
                nc.sync.dma_start(out=m1[:], in_=m1_d[:])
                bands[m] = (m0, m1)

            # activation biases must be [128,1] APs, not immediates
            bm25 = wpool.tile([128, 1], f32)
            bp10 = wpool.tile([128, 1], f32)
            bm2p5 = wpool.tile([128, 1], f32)
            nc.vector.memset(bm25[:], -25.0)
            nc.vector.memset(bp10[:], 10.0)
            nc.vector.memset(bm2p5[:], -2.5)

            strips = [(b, t) for b in range(B_PER_) for t in range(N_STRIPS_)]
            xt_tiles = {}

            def issue_load(b, t):
                """fp16 tile, partitions 0..M-1 = cells (rows r0..),
                partition M = bottom halo, M+1 = top halo.  gpsimd
                (SWDGE) DMA casts fp32->fp16 in flight."""
                r0 = t * STRIDE_
                M = min(STRIDE_, H_ - r0)
                xt = xpool.tile([KROWS, W_], f16, tag="xt")
                if r0 + M < H_:
                    nc.gpsimd.dma_start(
                        out=xt[0 : M + 1, :], in_=x_d[b, r0 : r0 + M + 1, :]
                    )
                else:
                    # last strip: bottom halo wraps to row 0
                    nc.gpsimd.dma_start(out=xt[0:M, :], in_=x_d[b, r0:H_, :])
                    nc.gpsimd.dma_start(out=xt[M : M + 1, :], in_=x_d[b, 0:1, :])
                rtop = (r0 - 1) % H_
                nc.gpsimd.dma_start(
                    out=xt[M + 1 : M + 2, :], in_=x_d[b, rtop : rtop + 1, :]
                )
                xt_tiles[(b, t)] = xt

            # Input DMA issued LOOKAHEAD strips ahead of compute so the Pool
            # sequencer's per-strip mul never gates the next strip's load
            # (the SWDGE trigger and the gpsimd mul share the Pool queue).
            LOOKAHEAD = 5
            next_load = 0

            for i, (b, t) in enumerate(strips):
                    while next_load < min(i + LOOKAHEAD, len(strips)):
                        issue_load(*strips[next_load])
                        next_load += 1
                    r0 = t * STRIDE_
                    M = min(STRIDE_, H_ - r0)  # output rows this strip
                    k = M + 2
                    m0, m1 = bands[M]
                    xt = xt_tiles.pop((b, t))

                    ps = ppool.tile([STRIDE_, W_], f32, tag="ps")
                    m0s = m0[:k, :M]
                    m1s = m1[:k, :M]

                    # Pre-touch: a 1x1 matmul absorbs the PSUM-release wait
                    # (Matmult carries at most ONE sync wait; without this,
                    # wait-merging couples strip t to strip t-1's consumers
                    # and serializes PE behind ACT/DVE).
                    nc.tensor.matmul(
                        ps[:1, 0:1], bm25[:1, :1], bm25[:1, :1],
                        start=True, stop=True,
                    )

                    # around = sum of 8 neighbors, accumulated in PSUM.
                    for nb in range(NBANKS_):
                        c0 = nb * 512
                        c1 = c0 + 512
                        # center column, vertical neighbors only
                        nc.tensor.matmul(
                            ps[:M, c0:c1], m0s, xt[:k, c0:c1],
                            start=True, stop=False,
                        )
                        # left-neighbor column: out col j += band @ x col j-1
                        if nb == 0:
                            nc.tensor.matmul(
                                ps[:M, 1:512], m1s, xt[:k, 0:511],
                                start=False, stop=False,
                            )
                            nc.tensor.matmul(
                                ps[:M, 0:1], m1s, xt[:k, W_ - 1 : W_],
                                start=False, stop=False,
                            )
                        else:
                            nc.tensor.matmul(
                                ps[:M, c0:c1], m1s, xt[:k, c0 - 1 : c1 - 1],
                                start=False, stop=False,
                            )
                        # right-neighbor column: out col j += band @ x col j+1
                        if nb == NBANKS_ - 1:
                            nc.tensor.matmul(
                                ps[:M, c0 : W_ - 1], m1s, xt[:k, c0 + 1 : W_],
                                start=False, stop=False,
                            )
                            nc.tensor.matmul(
                                ps[:M, W_ - 1 : W_], m1s, xt[:k, 0:1],
                                start=False, stop=True,
                            )
                        else:
                            nc.tensor.matmul(
                                ps[:M, c0:c1], m1s, xt[:k, c0 + 1 : c1 + 1],
                                start=False, stop=True,
                            )

                    # w = |around - 2.5|, split ScalarE / VectorE
                    wt = apool.tile([STRIDE_, W_], f16, tag="wt")
                    nc.scalar.activation(
                        wt[:M, 0:ABS_ACT_W], ps[:M, 0:ABS_ACT_W], AbsF,
                        bias=bm2p5[:M], scale=1.0,
                    )
                    nc.vector._custom_dve(
                        absshift, out=wt[:M, ABS_ACT_W:], in0=ps[:M, ABS_ACT_W:],
                        s0=-2.5,
                    )

                    # E2 = sigmoid(10*around - 25); E1 = sigmoid(10 - 10*w)
                    e2 = spool.tile([STRIDE_, W_], f16, tag="e2")
                    e1 = spool.tile([STRIDE_, W_], f16, tag="e1")
                    nc.scalar.activation(e2[:M], ps[:M], Sig, bias=bm25[:M], scale=10.0)
                    nc.scalar.activation(e1[:M], wt[:M], Sig, bias=bp10[:M], scale=-10.0)

                    # t = x + E2 - x*E2  (fused custom DVE op)
                    tt = tpool.tile([STRIDE_, W_], f16, tag="tt")
                    if USE_CUSTOM_BLEND:
                        bi = nc.vector._custom_dve(
                            blend1, out=tt[:M], in0=xt[:M, :], in1=e2[:M]
                        )
                        bi.ins.perf_max = 1
                    else:
                        mm = tpool.tile([STRIDE_, W_], f16, tag="mm")
                        nc.vector.tensor_mul(out=mm[:M], in0=xt[:M, :], in1=e2[:M])
                        nc.vector.tensor_sub(out=mm[:M], in0=e2[:M], in1=mm[:M])
                        nc.vector.tensor_add(out=tt[:M], in0=xt[:M, :], in1=mm[:M])

                    # out = E1 * t, split VectorE / GpSimd
                    o = opool.tile([STRIDE_, W_], f16, tag="o")
                    nc.vector.tensor_mul(
                        out=o[:M, 0:MUL_DVE_W], in0=e1[:M, 0:MUL_DVE_W],
                        in1=tt[:M, 0:MUL_DVE_W],
                    )
                    nc.gpsimd.tensor_mul(
                        out=o[:M, MUL_DVE_W:], in0=e1[:M, MUL_DVE_W:],
                        in1=tt[:M, MUL_DVE_W:],
                    )
                    nc.sync.dma_start(out=y_d[b, r0 : r0 + M, :], in_=o[:M])

    nc.compile()
    if (b_per, h, w, stride) == (B_PER, H, W, STRIDE):
        _cached_nc = nc
    return nc


def run(x, trace=False):
    """Run the SPMD kernel on 8 cores. Returns (out_fp32, BassKernelResults)."""
    from concourse.bass_utils import run_bass_kernel_spmd

    nc = _build()
    x = np.asarray(x, dtype=np.float32)
    assert x.shape == (B, H, W), x.shape
    in_maps = [{"x": x[B_PER * c : B_PER * (c + 1)]} for c in range(N_CORES)]
    res = run_bass_kernel_spmd(nc, in_maps, core_ids=list(range(N_CORES)), trace=trace)
    out = np.concatenate(
        [res.results[c]["y"].astype(np.float32) for c in range(N_CORES)], axis=0
    )
    return out, res


def kernel(x):
    out, _ = run(x, trace=False)
    return out
